# revision 1
# baseline (speedup 1.0000x reference)
"""HashEmbedding (hash -> gather -> sum-pool) on 8 TRN2 NeuronCores.

Strategy: batch-data-parallel (each core owns 512 of the 4096 batch rows
and a full copy of the [1M, 128] f32 table in its local HBM). Per-core
gather traffic (512*200 rows x 512 B = 52.4 MB) matches vocab-sharding
but needs no collectives.

The gather primitive with sim/HW parity on this stack is the ANT
`dma_gather` (gpsimd SWDGE CounterMachine, 16 SDMA engines). Its indices
are int16, so a single call can only address a 32768-row table window.
The host therefore hashes the ids (numpy uint32, exact) and sorts each
core's 102,400 (batch,slot) positions by window; the device executes 31
fixed-shape window gathers (capacity-padded) and pools with the
TensorEngine: per gathered chunk of 128 rows, a 0/1 assignment matrix
A[p, m] = (slot[p] == m) is built on the DVE via is_equal against an
iota, and psum[m, d] += A^T @ G accumulates the sum-pool. Padding slots
are -1 so padded rows match no column and contribute zero. Four PSUM
banks hold the four 128-row batch groups per core; results copy out
through SBUF.
"""

import sys

if "/opt/trn_rl_repo" not in sys.path:
    sys.path.insert(0, "/opt/trn_rl_repo")

import numpy as np

B, H, D, V = 4096, 200, 128, 1_000_000
NCORES = 8
BPC = B // NCORES              # 512 batch rows per core
NPASS = 4                      # batch groups of 128 rows (PSUM M limit)
WBITS = 15
W = 1 << WBITS                 # 32768-row window (int16 index limit)
NW = (V + W - 1) // W          # 31 windows
CAP = 1024                     # capacity per (window, pass); mu=839, sigma=28
CALL_IDX = NPASS * CAP         # 4096 indices per window call
CHUNKS = CALL_IDX // 128       # 32 matmul chunks per call

_cache: dict = {}


def _host_prep(x_core):
    """Hash + window-sort one core's ids -> (loc16 [NW,128,256] wrapped,
    slotf [NW,128,CHUNKS] f32)."""
    idx = (
        (x_core.astype(np.uint32).ravel() * np.uint32(2654435761))
        % np.uint32(V)
    ).astype(np.int32)                       # [BPC*H]
    b = np.repeat(np.arange(BPC, dtype=np.int32), H)
    win = idx >> WBITS
    loc = idx & (W - 1)
    grp = b >> 7                              # pass
    slot = b & 127

    bucket = win * NPASS + grp
    order = np.argsort(bucket, kind="stable")
    bs, ls, ss = bucket[order], loc[order], slot[order]
    counts = np.bincount(bucket, minlength=NW * NPASS)
    if counts.max() > CAP:
        raise RuntimeError(f"window bucket overflow: {counts.max()} > {CAP}")
    starts = np.zeros(NW * NPASS, dtype=np.int64)
    starts[1:] = np.cumsum(counts)[:-1]
    rank = np.arange(bs.size) - starts[bs]

    loc_arr = np.zeros((NW, NPASS, CAP), dtype=np.int16)
    slot_arr = np.full((NW, NPASS, CAP), -1.0, dtype=np.float32)
    loc_arr[bs // NPASS, bs % NPASS, rank] = ls.astype(np.int16)
    slot_arr[bs // NPASS, bs % NPASS, rank] = ss.astype(np.float32)

    flat_loc = loc_arr.reshape(NW, CALL_IDX)
    # SWDGE wrapped layout: position i at [partition i%16, col i//16],
    # replicated to all 8 Q7-core partition groups.
    wrapped = flat_loc.reshape(NW, CALL_IDX // 16, 16).transpose(0, 2, 1)
    loc16 = np.tile(wrapped, (1, 8, 1)).copy()            # [NW, 128, 256]
    # slot layout matching gather output: position i -> (p=i%128, c=i//128)
    slotf = (
        slot_arr.reshape(NW, CHUNKS, 128).transpose(0, 2, 1).copy()
    )                                                      # [NW, 128, CHUNKS]
    return loc16, slotf


def _build():
    import concourse.tile as tile
    from concourse import bacc, mybir

    i16, i32, f32 = mybir.dt.int16, mybir.dt.int32, mybir.dt.float32
    Alu = mybir.AluOpType

    nc = bacc.Bacc(
        "TRN2",
        target_bir_lowering=False,
        debug=False,
        enable_asserts=False,
        # SWDGE descriptor carveout: a dma_gather call of N descriptors
        # needs >= 32*N bytes here (HW-verified: 512 ok / 1024 crash at
        # the 16384 default; 1024 ok / 2048 crash at 32768).
        dynamic_dma_scratch_size=32768,
    )
    tb_ap = nc.dram_tensor("table", [NW * W, D], f32, kind="ExternalInput").ap()
    loc_ap = nc.dram_tensor(
        "loc16", [NW, 128, CALL_IDX // 16], i16, kind="ExternalInput"
    ).ap()
    slot_ap = nc.dram_tensor(
        "slotf", [NW, 128, CHUNKS], f32, kind="ExternalInput"
    ).ap()
    out_ap = nc.dram_tensor("out", [BPC, D], f32, kind="ExternalOutput").ap()

    with tile.TileContext(nc) as tc:
        with (
            tc.tile_pool(name="iop", bufs=1) as iop,
            tc.tile_pool(name="inp", bufs=3) as inp,
            tc.tile_pool(name="gp", bufs=3) as gp,
            tc.tile_pool(name="ap_", bufs=3) as ap_,
            tc.tile_pool(name="op", bufs=2) as op,
            tc.tile_pool(name="pp", bufs=1, space="PSUM") as pp,
        ):
            iota_i = iop.tile([128, 128], i32, name="iota_i")
            nc.gpsimd.iota(iota_i[:], [[1, 128]], base=0, channel_multiplier=0)
            iota_f = iop.tile([128, 128], f32, name="iota_f")
            nc.vector.tensor_copy(iota_f[:], iota_i[:])

            psums = [
                pp.tile([128, D], f32, name=f"ps{g}", tag=f"ps{g}")
                for g in range(NPASS)
            ]

            SUBC = CAP // 128                     # 8 chunks per (window, pass)
            for w in range(NW):
                lt = inp.tile([128, CALL_IDX // 16], i16, name="lt", tag="lt")
                nc.sync.dma_start(out=lt[:], in_=loc_ap[w])
                st = inp.tile([128, CHUNKS], f32, name="st", tag="st")
                nc.sync.dma_start(out=st[:], in_=slot_ap[w])

                A = ap_.tile([128, CHUNKS, 128], f32, name="A", tag="A")
                iota_bc = iota_f[:].unsqueeze(1).broadcast_to([128, CHUNKS, 128])
                st_bc = st[:].unsqueeze(2).broadcast_to([128, CHUNKS, 128])
                nc.vector.tensor_tensor(A[:], iota_bc, st_bc, Alu.is_equal)

                for grp in range(NPASS):
                    # one 1024-descriptor gather per (window, batch group)
                    g = gp.tile([128, SUBC, D], f32, name="g", tag="g")
                    nc.gpsimd.dma_gather(
                        g[:],
                        tb_ap[w * W : (w + 1) * W, :],
                        lt[:, grp * (CAP // 16) : (grp + 1) * (CAP // 16)],
                        CAP,
                        CAP,
                        D,
                    )
                    for c in range(SUBC):
                        nc.tensor.matmul(
                            psums[grp][:],
                            A[:, grp * SUBC + c, :],
                            g[:, c, :],
                            start=(w == 0 and c == 0),
                            stop=(w == NW - 1 and c == SUBC - 1),
                        )

            for grp in range(NPASS):
                outs = op.tile([128, D], f32, name="outs", tag="outs")
                nc.vector.tensor_copy(outs[:], psums[grp][:])
                nc.sync.dma_start(
                    out=out_ap[grp * 128 : (grp + 1) * 128, :], in_=outs[:]
                )

    nc.compile()
    return nc


def _run(x, table, trace=False):
    from concourse.bass_utils import run_bass_kernel_spmd

    if "nc" not in _cache:
        _cache["nc"] = _build()
    nc = _cache["nc"]

    x_np = np.asarray(x)
    # pad the table to NW*W rows so every gather window is a full 32768
    tb = np.zeros((NW * W, D), dtype=np.float32)
    tb[:V] = np.asarray(table, dtype=np.float32)
    in_maps = []
    for c in range(NCORES):
        loc16, slotf = _host_prep(x_np[c * BPC : (c + 1) * BPC])
        in_maps.append({"table": tb, "loc16": loc16, "slotf": slotf})
    res = run_bass_kernel_spmd(nc, in_maps, list(range(NCORES)), trace=trace)
    out = np.concatenate(
        [res.results[c]["out"] for c in range(NCORES)], axis=0
    ).astype(np.float32)
    return out, res


def kernel(x, table):
    out, _ = _run(x, table, trace=False)
    return out



# revision 18
# speedup vs baseline: 2.6883x; 2.6883x over previous
"""HashEmbedding (hash -> gather -> sum-pool) on 8 TRN2 NeuronCores.

Strategy: batch-data-parallel (each core owns 512 of the 4096 batch rows
and a full copy of the [1M, 128] table in its local HBM). Per-core gather
traffic matches vocab-sharding but needs no collectives.

The gather primitive is the ANT `dma_gather` (gpsimd SWDGE, int16 indices
-> 31 fixed 32768-row window gathers, capacity-padded). Profiling showed
the baseline was bound by Q7 descriptor generation (GpSimd engine 96%
busy, ~8.8us per 1024-index call): each dma_gather runs on ONE Q7 core
pair selected by queue_num, so this version round-robins the 124 calls
across all 4 SWDGE queues (4 core pairs working concurrently).

Other changes vs the serial-queue baseline:
- the table is stored bf16 (halves gather bytes; f32 PSUM accumulate
  keeps the pooling exact enough: rel err ~1e-3 << the 2e-2 gate);
- the sum-pool assignment matmul runs in bf16 (the f32 matmuls were
  456us of PE time, 4x the bf16 rate).
"""

import sys

if "/opt/trn_rl_repo" not in sys.path:
    sys.path.insert(0, "/opt/trn_rl_repo")

import ml_dtypes
import numpy as np

B, H, D, V = 4096, 200, 128, 1_000_000
NCORES = 8
BPC = B // NCORES              # 512 batch rows per core
NPASS = 4                      # batch groups of 128 rows (PSUM M limit)
WBITS = 15
W = 1 << WBITS                 # 32768-row window (int16 index limit)
NW = (V + W - 1) // W          # 31 windows
CAP = 1024                     # capacity per (window, pass); mu=839, max=925
CALL_IDX = NPASS * CAP         # 4096 indices per window
CHUNKS = CALL_IDX // 128       # 32 matmul chunks per window
SUBC = CAP // 128              # 8 chunks per (window, pass)
NQ = 4                         # SWDGE queues (Q7 core pairs)
GBUFS = 8                      # gather tile ring depth

_cache: dict = {}


def _host_prep(x_core):
    """Hash + window-sort one core's ids -> (loc16 [NW,128,256] wrapped,
    slotf [NW,128,CHUNKS] bf16). Padding slots get loc=-1 (the Q7 gather
    kernel trims trailing negatives: no descriptor, no DMA) and slot=-1
    (matches no assignment-matrix column: zero contribution)."""
    idx = (
        (x_core.astype(np.uint32).ravel() * np.uint32(2654435761))
        % np.uint32(V)
    ).astype(np.int32)                       # [BPC*H]
    b = np.repeat(np.arange(BPC, dtype=np.int32), H)
    win = idx >> WBITS
    loc = idx & (W - 1)
    grp = b >> 7                              # pass
    slot = b & 127

    bucket = win * NPASS + grp
    order = np.argsort(bucket, kind="stable")
    bs, ls, ss = bucket[order], loc[order], slot[order]
    counts = np.bincount(bucket, minlength=NW * NPASS)
    if counts.max() > CAP:
        raise RuntimeError(f"window bucket overflow: {counts.max()} > {CAP}")
    starts = np.zeros(NW * NPASS, dtype=np.int64)
    starts[1:] = np.cumsum(counts)[:-1]
    rank = np.arange(bs.size) - starts[bs]

    loc_arr = np.zeros((NW, NPASS, CAP), dtype=np.int16)
    slot_arr = np.full((NW, NPASS, CAP), -1.0, dtype=np.float32)
    loc_arr[bs // NPASS, bs % NPASS, rank] = ls.astype(np.int16)
    slot_arr[bs // NPASS, bs % NPASS, rank] = ss.astype(np.float32)

    flat_loc = loc_arr.reshape(NW, CALL_IDX)
    # SWDGE wrapped layout: position i at [partition i%16, col i//16],
    # replicated to all 8 Q7-core partition groups (any queue's pair
    # reads the copy on its own partitions).
    wrapped = flat_loc.reshape(NW, CALL_IDX // 16, 16).transpose(0, 2, 1)
    loc16 = np.tile(wrapped, (1, 8, 1)).copy()            # [NW, 128, 256]
    # slot layout matching gather output: position i -> (p=i%128, c=i//128)
    slotf = (
        slot_arr.reshape(NW, CHUNKS, 128)
        .transpose(0, 2, 1)
        .astype(ml_dtypes.bfloat16)
        .copy()
    )                                                      # [NW, 128, CHUNKS]
    return loc16, slotf


def _build():
    import concourse.tile as tile
    from concourse import bacc, mybir

    i16, i32 = mybir.dt.int16, mybir.dt.int32
    f32, bf16 = mybir.dt.float32, mybir.dt.bfloat16
    Alu = mybir.AluOpType

    nc = bacc.Bacc(
        "TRN2",
        target_bir_lowering=False,
        debug=False,
        enable_asserts=False,
        # SWDGE descriptor carveout: a dma_gather call of N descriptors
        # needs >= 32*N bytes here (HW-verified: 512 ok / 1024 crash at
        # the 16384 default; 1024 ok / 2048 crash at 32768). Ring state
        # is per queue (each queue pair's partitions hold its own rings).
        dynamic_dma_scratch_size=32768,
        num_swdge_queues=NQ,
    )
    tb_ap = nc.dram_tensor("table", [NW * W, D], bf16, kind="ExternalInput").ap()
    loc_ap = nc.dram_tensor(
        "loc16", [NW, 128, CALL_IDX // 16], i16, kind="ExternalInput"
    ).ap()
    slot_ap = nc.dram_tensor(
        "slotf", [NW, 128, CHUNKS], bf16, kind="ExternalInput"
    ).ap()
    out_ap = nc.dram_tensor("out", [BPC, D], f32, kind="ExternalOutput").ap()

    with tile.TileContext(nc) as tc:
        with (
            tc.tile_pool(name="iop", bufs=1) as iop,
            tc.tile_pool(name="inp", bufs=4) as inp,
            tc.tile_pool(name="gp", bufs=GBUFS) as gp,
            tc.tile_pool(name="ap_", bufs=3) as ap_,
            tc.tile_pool(name="op", bufs=2) as op,
            tc.tile_pool(name="pp", bufs=1, space="PSUM") as pp,
        ):
            iota_i = iop.tile([128, 128], i32, name="iota_i")
            nc.gpsimd.iota(iota_i[:], [[1, 128]], base=0, channel_multiplier=0)
            iota_b = iop.tile([128, 128], bf16, name="iota_b")
            nc.vector.tensor_copy(iota_b[:], iota_i[:])

            psums = [
                pp.tile([128, D], f32, name=f"ps{g}", tag=f"ps{g}")
                for g in range(NPASS)
            ]

            for w in range(NW):
                lt = inp.tile([128, CALL_IDX // 16], i16, name="lt", tag="lt")
                nc.sync.dma_start(out=lt[:], in_=loc_ap[w])
                st = inp.tile([128, CHUNKS], bf16, name="st", tag="st")
                nc.sync.dma_start(out=st[:], in_=slot_ap[w])

                A = ap_.tile([128, CHUNKS, 128], bf16, name="A", tag="A")
                iota_bc = iota_b[:].unsqueeze(1).broadcast_to([128, CHUNKS, 128])
                st_bc = st[:].unsqueeze(2).broadcast_to([128, CHUNKS, 128])
                nc.vector.tensor_tensor(A[:], iota_bc, st_bc, Alu.is_equal)

                for grp in range(NPASS):
                    # one gather per (window, batch group); queue_num picks
                    # the Q7 core pair, so 4 desc-gens run concurrently
                    g = gp.tile([128, SUBC, D], bf16, name="g", tag="g")
                    nc.gpsimd.dma_gather(
                        g[:],
                        tb_ap[w * W : (w + 1) * W, :],
                        lt[:, grp * (CAP // 16) : (grp + 1) * (CAP // 16)],
                        CAP,
                        CAP,
                        D,
                        queue_num=grp % NQ,
                    )
                    for c in range(SUBC):
                        nc.tensor.matmul(
                            psums[grp][:],
                            A[:, grp * SUBC + c, :],
                            g[:, c, :],
                            start=(w == 0 and c == 0),
                            stop=(w == NW - 1 and c == SUBC - 1),
                        )

            for grp in range(NPASS):
                outs = op.tile([128, D], f32, name="outs", tag="outs")
                nc.vector.tensor_copy(outs[:], psums[grp][:])
                nc.sync.dma_start(
                    out=out_ap[grp * 128 : (grp + 1) * 128, :], in_=outs[:]
                )

    nc.compile()
    return nc


def _prep_inputs(x, table):
    x_np = np.asarray(x)
    # pad the table to NW*W rows so every gather window is a full 32768
    tb = np.zeros((NW * W, D), dtype=ml_dtypes.bfloat16)
    tb[:V] = np.asarray(table).astype(ml_dtypes.bfloat16)
    in_maps = []
    for c in range(NCORES):
        loc16, slotf = _host_prep(x_np[c * BPC : (c + 1) * BPC])
        in_maps.append({"table": tb, "loc16": loc16, "slotf": slotf})
    return in_maps


def _run(x, table, trace=False):
    from concourse.bass_utils import run_bass_kernel_spmd

    if "nc" not in _cache:
        _cache["nc"] = _build()
    nc = _cache["nc"]

    in_maps = _prep_inputs(x, table)
    res = run_bass_kernel_spmd(nc, in_maps, list(range(NCORES)), trace=trace)
    out = np.concatenate(
        [res.results[c]["out"] for c in range(NCORES)], axis=0
    ).astype(np.float32)
    return out, res


def kernel(x, table):
    out, _ = _run(x, table, trace=False)
    return out


# revision 19
# speedup vs baseline: 3.7244x; 1.3854x over previous
"""HashEmbedding (hash -> gather -> sum-pool) on 8 TRN2 NeuronCores.

Strategy: batch-data-parallel (each core owns 512 of the 4096 batch rows
and a full copy of the [1M, 128] table in its local HBM). Per-core gather
traffic matches vocab-sharding but needs no collectives.

The gather primitive is the ANT `dma_gather` (gpsimd SWDGE, int16 indices
-> 31 fixed 32768-row window gathers, capacity-bounded). Perf history:
- baseline (single queue, f32): 1173us, GpSimd engine 96% busy at ~8.8us
  per 1024-index call -> Q7 descriptor generation bound.
- 4 SWDGE queues (each dma_gather runs on the Q7 core pair picked by
  queue_num, so 4 desc-gens run concurrently) + bf16 table and bf16
  pooling matmuls: 436us, GpSimd 86% / DMA engines ~80% busy.
- this version: per-call static num_idxs = max bucket count across the 8
  cores rounded up to 128 (the compile is specialized to the input's
  bucket histogram; ~12% fewer descriptors, matmuls, and assignment
  columns), packed chunk layout, single_packet=False so the SDMA engines
  can interleave packets across the 4 queue rings.

Pooling: per gathered chunk of 128 rows, a 0/1 assignment matrix
A[p, m] = (slot[p] == m) is built on the DVE via is_equal against an
iota, and psum[m, d] += A^T @ G accumulates the sum-pool in f32 PSUM.
Padding slots are -1 so they match no column and contribute zero.
"""

import sys

if "/opt/trn_rl_repo" not in sys.path:
    sys.path.insert(0, "/opt/trn_rl_repo")

import ml_dtypes
import numpy as np

B, H, D, V = 4096, 200, 128, 1_000_000
NCORES = 8
BPC = B // NCORES              # 512 batch rows per core
NPASS = 4                      # batch groups of 128 rows (PSUM M limit)
WBITS = 15
W = 1 << WBITS                 # 32768-row window (int16 index limit)
NW = (V + W - 1) // W          # 31 windows
CAP = 1024                     # hard capacity per (window, pass) bucket
CALL_IDX = NPASS * CAP         # flat index layout stride per window
CHUNKS = CALL_IDX // 128       # max matmul chunks per window
NQ = 4                         # SWDGE queues (Q7 core pairs)
GBUFS = 8                      # gather tile ring depth

_cache: dict = {}


def _bucket_counts(x_core):
    """Per-(window, pass) bucket histogram for one core. Also returns the
    (idx, b) decomposition reused by _host_prep."""
    idx = (
        (x_core.astype(np.uint32).ravel() * np.uint32(2654435761))
        % np.uint32(V)
    ).astype(np.int32)                       # [BPC*H]
    b = np.repeat(np.arange(BPC, dtype=np.int32), H)
    bucket = (idx >> WBITS) * NPASS + (b >> 7)
    counts = np.bincount(bucket, minlength=NW * NPASS)
    return idx, b, bucket, counts


def _host_prep(idx, b, bucket, n128):
    """Window-sort one core's positions -> (loc16 [NW,128,256] wrapped,
    slotf [NW,128,CHUNKS] bf16 with per-window used chunks packed
    contiguously). n128 [NW, NPASS]: static per-call index counts (>= this
    core's bucket counts); padding gathers row 0 with slot=-1 (matches no
    assignment column -> contributes zero)."""
    loc = idx & (W - 1)
    slot = b & 127

    order = np.argsort(bucket, kind="stable")
    bs, ls, ss = bucket[order], loc[order], slot[order]
    counts = np.bincount(bucket, minlength=NW * NPASS)
    starts = np.zeros(NW * NPASS, dtype=np.int64)
    starts[1:] = np.cumsum(counts)[:-1]
    rank = np.arange(bs.size) - starts[bs]

    loc_arr = np.zeros((NW, NPASS, CAP), dtype=np.int16)
    slot_arr = np.full((NW, NPASS, CAP), -1.0, dtype=np.float32)
    loc_arr[bs // NPASS, bs % NPASS, rank] = ls.astype(np.int16)
    slot_arr[bs // NPASS, bs % NPASS, rank] = ss.astype(np.float32)

    flat_loc = loc_arr.reshape(NW, CALL_IDX)
    # SWDGE wrapped layout: position i at [partition i%16, col i//16],
    # replicated to all 8 Q7-core partition groups (any queue's pair
    # reads the copy on its own partitions). Call (w, grp) reads cols
    # [grp*64, grp*64 + n128[w,grp]//16).
    wrapped = flat_loc.reshape(NW, CALL_IDX // 16, 16).transpose(0, 2, 1)
    loc16 = np.tile(wrapped, (1, 8, 1)).copy()            # [NW, 128, 256]

    # slot layout matching gather output (position i -> p=i%128, c=i//128),
    # with each window's used chunks packed contiguously:
    # col off[w,grp]+c holds call (w,grp) chunk c.
    cw = n128 // 128                                       # [NW, NPASS]
    slotf = np.full((NW, 128, CHUNKS), -1.0, dtype=np.float32)
    for w in range(NW):
        off = 0
        for g in range(NPASS):
            k = cw[w, g]
            chunks = slot_arr[w, g, : k * 128].reshape(k, 128).T  # [128, k]
            slotf[w, :, off : off + k] = chunks
            off += k
    return loc16, slotf.astype(ml_dtypes.bfloat16)


def _build(n128):
    import concourse.tile as tile
    from concourse import bacc, mybir

    i16, i32 = mybir.dt.int16, mybir.dt.int32
    f32, bf16 = mybir.dt.float32, mybir.dt.bfloat16
    Alu = mybir.AluOpType

    cw = n128 // 128                   # [NW, NPASS] chunks per call
    tcw = cw.sum(axis=1)               # [NW] used chunks per window

    nc = bacc.Bacc(
        "TRN2",
        target_bir_lowering=False,
        debug=False,
        enable_asserts=False,
        # SWDGE descriptor carveout: a dma_gather call of N descriptors
        # needs >= 32*N bytes here. Ring state is per queue (each queue
        # pair's partitions hold its own rings).
        dynamic_dma_scratch_size=32768,
        num_swdge_queues=NQ,
    )
    tb_ap = nc.dram_tensor("table", [NW * W, D], bf16, kind="ExternalInput").ap()
    loc_ap = nc.dram_tensor(
        "loc16", [NW, 128, CALL_IDX // 16], i16, kind="ExternalInput"
    ).ap()
    slot_ap = nc.dram_tensor(
        "slotf", [NW, 128, CHUNKS], bf16, kind="ExternalInput"
    ).ap()
    out_ap = nc.dram_tensor("out", [BPC, D], f32, kind="ExternalOutput").ap()

    with tile.TileContext(nc) as tc:
        with (
            tc.tile_pool(name="iop", bufs=1) as iop,
            tc.tile_pool(name="inp", bufs=4) as inp,
            tc.tile_pool(name="gp", bufs=GBUFS) as gp,
            tc.tile_pool(name="ap_", bufs=3) as ap_,
            tc.tile_pool(name="op", bufs=2) as op,
            tc.tile_pool(name="pp", bufs=1, space="PSUM") as pp,
        ):
            iota_i = iop.tile([128, 128], i32, name="iota_i")
            nc.gpsimd.iota(iota_i[:], [[1, 128]], base=0, channel_multiplier=0)
            iota_b = iop.tile([128, 128], bf16, name="iota_b")
            nc.vector.tensor_copy(iota_b[:], iota_i[:])

            psums = [
                pp.tile([128, D], f32, name=f"ps{g}", tag=f"ps{g}")
                for g in range(NPASS)
            ]

            for w in range(NW):
                lt = inp.tile([128, CALL_IDX // 16], i16, name="lt", tag="lt")
                nc.sync.dma_start(out=lt[:], in_=loc_ap[w])
                st = inp.tile([128, CHUNKS], bf16, name="st", tag="st")
                nc.sync.dma_start(out=st[:], in_=slot_ap[w])

                t = int(tcw[w])
                A = ap_.tile([128, CHUNKS, 128], bf16, name="A", tag="A")
                iota_bc = iota_b[:].unsqueeze(1).broadcast_to([128, t, 128])
                st_bc = st[:, :t].unsqueeze(2).broadcast_to([128, t, 128])
                nc.vector.tensor_tensor(A[:, :t], iota_bc, st_bc, Alu.is_equal)

                off = 0
                for grp in range(NPASS):
                    n = int(n128[w, grp])
                    k = int(cw[w, grp])
                    # one gather per (window, batch group); queue_num picks
                    # the Q7 core pair, so 4 desc-gens run concurrently
                    g = gp.tile([128, k, D], bf16, name="g", tag="g")
                    nc.gpsimd.dma_gather(
                        g[:],
                        tb_ap[w * W : (w + 1) * W, :],
                        lt[:, grp * (CAP // 16) : grp * (CAP // 16) + n // 16],
                        n,
                        n,
                        D,
                        queue_num=grp % NQ,
                        single_packet=False,
                    )
                    for c in range(k):
                        nc.tensor.matmul(
                            psums[grp][:],
                            A[:, off + c, :],
                            g[:, c, :],
                            start=(w == 0 and c == 0),
                            stop=(w == NW - 1 and c == k - 1),
                        )
                    off += k

            for grp in range(NPASS):
                outs = op.tile([128, D], f32, name="outs", tag="outs")
                nc.vector.tensor_copy(outs[:], psums[grp][:])
                nc.sync.dma_start(
                    out=out_ap[grp * 128 : (grp + 1) * 128, :], in_=outs[:]
                )

    nc.compile()
    return nc


def _prep_inputs(x, table):
    x_np = np.asarray(x)
    per_core = [
        _bucket_counts(x_np[c * BPC : (c + 1) * BPC]) for c in range(NCORES)
    ]
    counts_max = np.max([pc[3] for pc in per_core], axis=0)
    if counts_max.max() > CAP:
        raise RuntimeError(f"window bucket overflow: {counts_max.max()} > {CAP}")
    n128 = (
        ((counts_max.reshape(NW, NPASS) + 127) // 128) * 128
    ).astype(np.int64)

    # pad the table to NW*W rows so every gather window is a full 32768
    tb = np.zeros((NW * W, D), dtype=ml_dtypes.bfloat16)
    tb[:V] = np.asarray(table).astype(ml_dtypes.bfloat16)
    in_maps = []
    for c in range(NCORES):
        idx, b, bucket, _ = per_core[c]
        loc16, slotf = _host_prep(idx, b, bucket, n128)
        in_maps.append({"table": tb, "loc16": loc16, "slotf": slotf})
    return n128, in_maps


def _run(x, table, trace=False):
    from concourse.bass_utils import run_bass_kernel_spmd

    n128, in_maps = _prep_inputs(x, table)
    key = n128.tobytes()
    if _cache.get("key") != key:
        _cache["nc"] = _build(n128)
        _cache["key"] = key
    nc = _cache["nc"]

    res = run_bass_kernel_spmd(nc, in_maps, list(range(NCORES)), trace=trace)
    out = np.concatenate(
        [res.results[c]["out"] for c in range(NCORES)], axis=0
    ).astype(np.float32)
    return out, res


def kernel(x, table):
    out, _ = _run(x, table, trace=False)
    return out
